# revision 1
# baseline (speedup 1.0000x reference)
"""Chamfer loss Trainium2 kernel.

Problem: B=8 batches of pred[4096,3] vs tgt[4096,3] point clouds.
chamfer = mean_n min_m ||p_n - t_m|| + mean_m min_n ||p_n - t_m||

Sharding: one batch element per NeuronCore (8 cores, SPMD).

Math: sqrt is monotonic, so mins are taken over *squared* distances and
only the final [4096] min-vectors get sqrt'd.  The squared distance
p2 + t2 - 2<p,t> is computed by a single K=5 augmented matmul:
  lhsT rows: [-2px, -2py, -2pz, 1, p2],  rhs rows: [tx, ty, tz, t2, 1]
so the PE writes sq[n,m] tiles straight into PSUM.  With K=5 the PE
array is addressed in 32-row strips (tile_position) so 4 matmuls run
concurrently.  DVE reduce_min drains PSUM; both orientations (row-min
and col-min) are separate matmul passes so each min is a free-axis
reduction.  Final sqrt+sum on device; the last [128,2]-per-core
reduction and the cross-core mean happen on host.
"""

import os
import numpy as np

B = 8
N = 4096  # pred points per batch
M = 4096  # tgt points per batch
D = 3
K = 5     # augmented contraction dim
P = 128   # partition block (rows per n-block)
F = 512   # matmul moving free dim (fp32 max / one PSUM bank)
NBLK = N // P   # 32
NCHK = M // F   # 8

_CACHE = {}


def _build_bass():
    import concourse.tile as tile
    from concourse import bacc, mybir

    f32 = mybir.dt.float32
    nc = bacc.Bacc(None, target_bir_lowering=False)

    wA = nc.dram_tensor("wA", [K, N], f32, kind="ExternalInput")
    rA = nc.dram_tensor("rA", [K, M], f32, kind="ExternalInput")
    wB = nc.dram_tensor("wB", [K, M], f32, kind="ExternalInput")
    rB = nc.dram_tensor("rB", [K, N], f32, kind="ExternalInput")
    out = nc.dram_tensor("out", [P, 2], f32, kind="ExternalOutput")

    with tile.TileContext(nc) as tc:
        with (
            tc.tile_pool(name="inp", bufs=1) as inp_pool,
            tc.tile_pool(name="psum", bufs=2, space="PSUM") as psum_pool,
            tc.tile_pool(name="acc", bufs=1) as acc_pool,
        ):
            sums = acc_pool.tile([P, 2], f32, name="sums")
            for oi, (wd, rd) in enumerate([(wA, rA), (wB, rB)]):
                Wt = inp_pool.tile([P, N], f32, name=f"Wt{oi}")
                Rt = inp_pool.tile([P, M], f32, name=f"Rt{oi}")
                # replicate the K=5 augmented rows into the 4 PE row strips
                for s in range(4):
                    nc.sync.dma_start(Wt[32 * s:32 * s + K, :], wd[:, :])
                    nc.sync.dma_start(Rt[32 * s:32 * s + K, :], rd[:, :])

                # per-block running mins: two halves per block
                rowmin2 = acc_pool.tile([P, 2 * NBLK], f32, name=f"rowmin2_{oi}")
                for i in range(NBLK):
                    for h in range(2):
                        ps = psum_pool.tile([P, 4 * F], f32, tag="ps")
                        for j in range(4):
                            c = h * 4 + j
                            s = c % 4
                            nc.tensor.matmul(
                                ps[:, j * F:(j + 1) * F],
                                Wt[32 * s:32 * s + K, i * P:(i + 1) * P],
                                Rt[32 * s:32 * s + K, c * F:(c + 1) * F],
                                start=True,
                                stop=True,
                                tile_position=(32 * s, 0),
                            )
                        nc.vector.tensor_reduce(
                            rowmin2[:, 2 * i + h:2 * i + h + 1],
                            ps[:, :],
                            axis=mybir.AxisListType.X,
                            op=mybir.AluOpType.min,
                        )
                # combine half-mins, clamp, sqrt, per-partition sum
                pairs = rowmin2.rearrange("p (i two) -> p i two", two=2)
                rowmin = acc_pool.tile([P, NBLK], f32, name=f"rowmin_{oi}")
                nc.vector.tensor_tensor(
                    rowmin[:, :], pairs[:, :, 0], pairs[:, :, 1],
                    op=mybir.AluOpType.min,
                )
                nc.vector.tensor_scalar_max(rowmin[:, :], rowmin[:, :], 0.0)
                dist = acc_pool.tile([P, NBLK], f32, name=f"dist_{oi}")
                nc.scalar.sqrt(dist[:, :], rowmin[:, :])
                nc.vector.tensor_reduce(
                    sums[:, oi:oi + 1], dist[:, :],
                    axis=mybir.AxisListType.X, op=mybir.AluOpType.add,
                )
            nc.sync.dma_start(out[:, :], sums[:, :])

    nc.finalize()
    return nc


def _get_nc():
    if "nc" not in _CACHE:
        _CACHE["nc"] = _build_bass()
    return _CACHE["nc"]


def _augment(pts_w, pts_r):
    """Build (lhsT, rhs) aug matrices: sq = lhsT.T @ rhs."""
    ones_w = np.ones(pts_w.shape[0], np.float32)
    w2 = (pts_w * pts_w).sum(-1)
    r2 = (pts_r * pts_r).sum(-1)
    ones_r = np.ones(pts_r.shape[0], np.float32)
    lhsT = np.ascontiguousarray(
        np.stack([-2.0 * pts_w[:, 0], -2.0 * pts_w[:, 1], -2.0 * pts_w[:, 2],
                  ones_w, w2]).astype(np.float32))
    rhs = np.ascontiguousarray(
        np.stack([pts_r[:, 0], pts_r[:, 1], pts_r[:, 2], r2,
                  ones_r]).astype(np.float32))
    return lhsT, rhs


def _in_maps(predicted_points, target_points):
    maps = []
    for b in range(B):
        p = np.asarray(predicted_points[b], np.float32)
        t = np.asarray(target_points[b], np.float32)
        wA, rA = _augment(p, t)
        wB, rB = _augment(t, p)
        maps.append({"wA": wA, "rA": rA, "wB": wB, "rB": rB})
    return maps


def kernel(predicted_points, target_points):
    from concourse.bass_utils import run_bass_kernel_spmd

    nc = _get_nc()
    in_maps = _in_maps(predicted_points, target_points)
    trace = bool(int(os.environ.get("CHAMFER_TRACE", "0")))
    res = run_bass_kernel_spmd(
        nc, in_maps, core_ids=list(range(B)),
        trace=trace, trace_cores=[0] if trace else None,
    )
    _CACHE["last_result"] = res
    tot_a = 0.0
    tot_b = 0.0
    for b in range(B):
        o = res.results[b]["out"].astype(np.float64)
        tot_a += o[:, 0].sum()
        tot_b += o[:, 1].sum()
    return np.float32(tot_a / (B * N) + tot_b / (B * M))


# revision 3
# speedup vs baseline: 1.0546x; 1.0546x over previous
"""Chamfer loss Trainium2 kernel.

Problem: B=8 batches of pred[4096,3] vs tgt[4096,3] point clouds.
chamfer = mean_n min_m ||p_n - t_m|| + mean_m min_n ||p_n - t_m||

Sharding: one batch element per NeuronCore (8 cores, SPMD).

Math: sqrt is monotonic, so mins are taken over *squared* distances and
only the final [4096] min-vectors get sqrt'd.  The squared distance
p2 + t2 - 2<p,t> is computed by a single K=5 augmented matmul:
  lhsT rows: [-2px, -2py, -2pz, 1, p2],  rhs rows: [tx, ty, tz, t2, 1]
so the PE writes sq[n,m] tiles straight into PSUM.  With K=5 the PE
array is addressed in 32-row strips (tile_position) so 4 matmuls run
concurrently.  DVE reduce_min drains PSUM; both orientations (row-min
and col-min) are separate matmul passes so each min is a free-axis
reduction.  Final sqrt+sum on device; the last [128,2]-per-core
reduction and the cross-core mean happen on host.
"""

import os
import numpy as np

B = 8
N = 4096  # pred points per batch
M = 4096  # tgt points per batch
D = 3
K = 5     # augmented contraction dim
P = 128   # partition block (rows per n-block)
F = 512   # matmul moving free dim (fp32 max / one PSUM bank)
NBLK = N // P   # 32
NCHK = M // F   # 8

_CACHE = {}


def _build_bass():
    import concourse.tile as tile
    from concourse import bacc, mybir

    f32 = mybir.dt.float32
    nc = bacc.Bacc(None, target_bir_lowering=False)

    f32r = mybir.dt.float32r
    wA = nc.dram_tensor("wA", [K, N], f32r, kind="ExternalInput")
    rA = nc.dram_tensor("rA", [K, M], f32r, kind="ExternalInput")
    wB = nc.dram_tensor("wB", [K, M], f32r, kind="ExternalInput")
    rB = nc.dram_tensor("rB", [K, N], f32r, kind="ExternalInput")
    out = nc.dram_tensor("out", [P, 2], f32, kind="ExternalOutput")

    with tile.TileContext(nc) as tc:
        with (
            tc.tile_pool(name="inp", bufs=1) as inp_pool,
            tc.tile_pool(name="psum", bufs=2, space="PSUM") as psum_pool,
            tc.tile_pool(name="acc", bufs=1) as acc_pool,
        ):
            sums = acc_pool.tile([P, 2], f32, name="sums")
            for oi, (wd, rd) in enumerate([(wA, rA), (wB, rB)]):
                Wt = inp_pool.tile([P, N], f32r, name=f"Wt{oi}")
                Rt = inp_pool.tile([P, M], f32r, name=f"Rt{oi}")
                # replicate the K=5 augmented rows into the 4 PE row strips
                for s in range(4):
                    nc.sync.dma_start(Wt[32 * s:32 * s + K, :], wd[:, :])
                    nc.sync.dma_start(Rt[32 * s:32 * s + K, :], rd[:, :])

                # per-block running mins: two halves per block
                rowmin2 = acc_pool.tile([P, 2 * NBLK], f32, name=f"rowmin2_{oi}")
                for i in range(NBLK):
                    for h in range(2):
                        ps = psum_pool.tile([P, 4 * F], f32, tag="ps")
                        for j in range(4):
                            c = h * 4 + j
                            s = c % 4
                            # float32r: fp32 bits via the replicated/transpose
                            # weight path — 1 cyc/col (vs 4 for plain fp32)
                            # when the moving free dim is >= 256
                            nc.tensor.matmul(
                                ps[:, j * F:(j + 1) * F],
                                Wt[32 * s:32 * s + K, i * P:(i + 1) * P],
                                Rt[32 * s:32 * s + K, c * F:(c + 1) * F],
                                start=True,
                                stop=True,
                                tile_position=(32 * s, 0),
                            )
                        nc.vector.tensor_reduce(
                            rowmin2[:, 2 * i + h:2 * i + h + 1],
                            ps[:, :],
                            axis=mybir.AxisListType.X,
                            op=mybir.AluOpType.min,
                        )
                # combine half-mins, clamp, sqrt, per-partition sum
                pairs = rowmin2.rearrange("p (i two) -> p i two", two=2)
                rowmin = acc_pool.tile([P, NBLK], f32, name=f"rowmin_{oi}")
                nc.vector.tensor_tensor(
                    rowmin[:, :], pairs[:, :, 0], pairs[:, :, 1],
                    op=mybir.AluOpType.min,
                )
                nc.vector.tensor_scalar_max(rowmin[:, :], rowmin[:, :], 0.0)
                dist = acc_pool.tile([P, NBLK], f32, name=f"dist_{oi}")
                nc.scalar.sqrt(dist[:, :], rowmin[:, :])
                nc.vector.tensor_reduce(
                    sums[:, oi:oi + 1], dist[:, :],
                    axis=mybir.AxisListType.X, op=mybir.AluOpType.add,
                )
            nc.sync.dma_start(out[:, :], sums[:, :])

    nc.finalize()
    return nc


def _get_nc():
    if "nc" not in _CACHE:
        _CACHE["nc"] = _build_bass()
    return _CACHE["nc"]


def _augment(pts_w, pts_r):
    """Build (lhsT, rhs) aug matrices: sq = lhsT.T @ rhs."""
    ones_w = np.ones(pts_w.shape[0], np.float32)
    w2 = (pts_w * pts_w).sum(-1)
    r2 = (pts_r * pts_r).sum(-1)
    ones_r = np.ones(pts_r.shape[0], np.float32)
    lhsT = np.ascontiguousarray(
        np.stack([-2.0 * pts_w[:, 0], -2.0 * pts_w[:, 1], -2.0 * pts_w[:, 2],
                  ones_w, w2]).astype(np.float32))
    rhs = np.ascontiguousarray(
        np.stack([pts_r[:, 0], pts_r[:, 1], pts_r[:, 2], r2,
                  ones_r]).astype(np.float32))
    return lhsT, rhs


def _in_maps(predicted_points, target_points):
    maps = []
    for b in range(B):
        p = np.asarray(predicted_points[b], np.float32)
        t = np.asarray(target_points[b], np.float32)
        wA, rA = _augment(p, t)
        wB, rB = _augment(t, p)
        maps.append({"wA": wA, "rA": rA, "wB": wB, "rB": rB})
    return maps


def kernel(predicted_points, target_points):
    from concourse.bass_utils import run_bass_kernel_spmd

    nc = _get_nc()
    in_maps = _in_maps(predicted_points, target_points)
    trace = bool(int(os.environ.get("CHAMFER_TRACE", "0")))
    res = run_bass_kernel_spmd(
        nc, in_maps, core_ids=list(range(B)),
        trace=trace, trace_cores=[0] if trace else None,
    )
    _CACHE["last_result"] = res
    tot_a = 0.0
    tot_b = 0.0
    for b in range(B):
        o = res.results[b]["out"].astype(np.float64)
        tot_a += o[:, 0].sum()
        tot_b += o[:, 1].sum()
    return np.float32(tot_a / (B * N) + tot_b / (B * M))


# revision 6
# speedup vs baseline: 1.1832x; 1.1219x over previous
"""Chamfer loss Trainium2 kernel.

Problem: B=8 batches of pred[4096,3] vs tgt[4096,3] point clouds.
chamfer = mean_n min_m ||p_n - t_m|| + mean_m min_n ||p_n - t_m||

Sharding: one batch element per NeuronCore (8 cores, SPMD).

Math: sqrt is monotonic, so mins are taken over *squared* distances and
only the final [4096] min-vectors get sqrt'd.  The squared distance
p2 + t2 - 2<p,t> is computed by a single K=5 augmented matmul:
  lhsT rows: [-2px, -2py, -2pz, 1, p2],  rhs rows: [tx, ty, tz, t2, 1]
so the PE writes sq[n,m] tiles straight into PSUM.  With K=5 the PE
array is addressed in 32-row strips (tile_position) so 4 matmuls run
concurrently.  DVE reduce_min drains PSUM; both orientations (row-min
and col-min) are separate matmul passes so each min is a free-axis
reduction.  Final sqrt+sum on device; the last [128,2]-per-core
reduction and the cross-core mean happen on host.
"""

import os
import numpy as np

B = 8
N = 4096  # pred points per batch
M = 4096  # tgt points per batch
D = 3
K = 5     # augmented contraction dim
P = 128   # partition block (rows per n-block)
F = 512   # matmul moving free dim (fp32 max / one PSUM bank)
NBLK = N // P   # 32
NCHK = M // F   # 8

_CACHE = {}


def _build_bass():
    import concourse.tile as tile
    from concourse import bacc, mybir

    f32 = mybir.dt.float32
    nc = bacc.Bacc(None, target_bir_lowering=False)

    f32r = mybir.dt.float32r
    wA = nc.dram_tensor("wA", [K, N], f32r, kind="ExternalInput")
    rA = nc.dram_tensor("rA", [K, M], f32r, kind="ExternalInput")
    wB = nc.dram_tensor("wB", [K, M], f32r, kind="ExternalInput")
    rB = nc.dram_tensor("rB", [K, N], f32r, kind="ExternalInput")
    out = nc.dram_tensor("out", [P, 2], f32, kind="ExternalOutput")

    f16 = mybir.dt.float16
    with tile.TileContext(nc) as tc:
        with (
            tc.tile_pool(name="inp", bufs=1) as inp_pool,
            tc.tile_pool(name="psum", bufs=2, space="PSUM") as psum_pool,
            tc.tile_pool(name="acc", bufs=1) as acc_pool,
            tc.tile_pool(name="cvt", bufs=3) as cvt_pool,
            tc.tile_pool(name="scr", bufs=2) as scr_pool,
        ):
            sums = acc_pool.tile([P, 2], f32, name="sums")
            for oi, (wd, rd) in enumerate([(wA, rA), (wB, rB)]):
                Wt = inp_pool.tile([P, N], f32r, name=f"Wt{oi}")
                Rt = inp_pool.tile([P, M], f32r, name=f"Rt{oi}")
                # replicate the K=5 augmented rows into the 4 PE row strips
                for s in range(4):
                    nc.sync.dma_start(Wt[32 * s:32 * s + K, :], wd[:, :])
                    nc.sync.dma_start(Rt[32 * s:32 * s + K, :], rd[:, :])

                # Per-block drain split (8 chunks of 512 per block):
                #  - DVE reduce_min's the first SD elems straight from PSUM
                #  - ACT (otherwise idle; it cannot min but CAN drain PSUM)
                #    converts the rest to fp16 in SBUF
                #  - DVE finishes the fp16 side with a TT-min tree at
                #    2 results/cycle, batched over G blocks per instruction
                #    to amortize per-op overhead
                SD = 768              # direct-path elems per block
                SC = 8 * F - SD       # 3328 fp16-path elems per block
                G = 4                 # blocks per batched tree
                rowdir = acc_pool.tile([P, NBLK], f32, name=f"rowdir_{oi}")
                rowm16 = acc_pool.tile([P, NBLK], f16, name=f"rowm16_{oi}")
                for g in range(NBLK // G):
                    cvt = cvt_pool.tile([P, G, SC], f16, tag="cvt")
                    for bi in range(G):
                        i = g * G + bi
                        pss = []
                        for h in range(2):
                            ps = psum_pool.tile([P, 4 * F], f32, tag="ps")
                            for j in range(4):
                                c = h * 4 + j
                                s = c % 4
                                # float32r: fp32 bits via the replicated
                                # weight path — much faster than plain fp32
                                # when the moving free dim is >= 256
                                nc.tensor.matmul(
                                    ps[:, j * F:(j + 1) * F],
                                    Wt[32 * s:32 * s + K, i * P:(i + 1) * P],
                                    Rt[32 * s:32 * s + K, c * F:(c + 1) * F],
                                    start=True,
                                    stop=True,
                                    tile_position=(32 * s, 0),
                                )
                            pss.append(ps)
                        nc.vector.tensor_reduce(
                            rowdir[:, i:i + 1],
                            pss[0][:, 0:SD],
                            axis=mybir.AxisListType.X,
                            op=mybir.AluOpType.min,
                        )
                        nc.scalar.copy(cvt[:, bi, 0:4 * F - SD],
                                       pss[0][:, SD:4 * F])
                        nc.scalar.copy(cvt[:, bi, 4 * F - SD:SC], pss[1][:, :])
                    # batched fp16 min tree: SC -> SC/2 -> ... -> SC/16 -> 1
                    prev = cvt
                    w = SC
                    for lvl in range(4):
                        w //= 2
                        nxt = scr_pool.tile([P, G, w], f16, tag=f"l{lvl}")
                        nc.vector.tensor_tensor(
                            nxt[:, :, :], prev[:, :, 0:w], prev[:, :, w:2 * w],
                            op=mybir.AluOpType.min,
                        )
                        prev = nxt
                    nc.vector.tensor_reduce(
                        rowm16[:, g * G:(g + 1) * G], prev[:, :, :],
                        axis=mybir.AxisListType.X, op=mybir.AluOpType.min,
                    )
                # combine fp32/fp16 paths, clamp, sqrt, per-partition sum
                rowm16f = acc_pool.tile([P, NBLK], f32, name=f"rowm16f_{oi}")
                nc.vector.tensor_copy(rowm16f[:, :], rowm16[:, :])
                rowmin = acc_pool.tile([P, NBLK], f32, name=f"rowmin_{oi}")
                nc.vector.tensor_tensor(
                    rowmin[:, :], rowdir[:, :], rowm16f[:, :],
                    op=mybir.AluOpType.min,
                )
                nc.vector.tensor_scalar_max(rowmin[:, :], rowmin[:, :], 0.0)
                dist = acc_pool.tile([P, NBLK], f32, name=f"dist_{oi}")
                nc.scalar.sqrt(dist[:, :], rowmin[:, :])
                nc.vector.tensor_reduce(
                    sums[:, oi:oi + 1], dist[:, :],
                    axis=mybir.AxisListType.X, op=mybir.AluOpType.add,
                )
            nc.sync.dma_start(out[:, :], sums[:, :])

    nc.finalize()
    return nc


def _get_nc():
    if "nc" not in _CACHE:
        _CACHE["nc"] = _build_bass()
    return _CACHE["nc"]


def _augment(pts_w, pts_r):
    """Build (lhsT, rhs) aug matrices: sq = lhsT.T @ rhs."""
    ones_w = np.ones(pts_w.shape[0], np.float32)
    w2 = (pts_w * pts_w).sum(-1)
    r2 = (pts_r * pts_r).sum(-1)
    ones_r = np.ones(pts_r.shape[0], np.float32)
    lhsT = np.ascontiguousarray(
        np.stack([-2.0 * pts_w[:, 0], -2.0 * pts_w[:, 1], -2.0 * pts_w[:, 2],
                  ones_w, w2]).astype(np.float32))
    rhs = np.ascontiguousarray(
        np.stack([pts_r[:, 0], pts_r[:, 1], pts_r[:, 2], r2,
                  ones_r]).astype(np.float32))
    return lhsT, rhs


def _in_maps(predicted_points, target_points):
    maps = []
    for b in range(B):
        p = np.asarray(predicted_points[b], np.float32)
        t = np.asarray(target_points[b], np.float32)
        wA, rA = _augment(p, t)
        wB, rB = _augment(t, p)
        maps.append({"wA": wA, "rA": rA, "wB": wB, "rB": rB})
    return maps


def kernel(predicted_points, target_points):
    from concourse.bass_utils import run_bass_kernel_spmd

    nc = _get_nc()
    in_maps = _in_maps(predicted_points, target_points)
    trace = bool(int(os.environ.get("CHAMFER_TRACE", "0")))
    res = run_bass_kernel_spmd(
        nc, in_maps, core_ids=list(range(B)),
        trace=trace, trace_cores=[0] if trace else None,
    )
    _CACHE["last_result"] = res
    tot_a = 0.0
    tot_b = 0.0
    for b in range(B):
        o = res.results[b]["out"].astype(np.float64)
        tot_a += o[:, 0].sum()
        tot_b += o[:, 1].sum()
    return np.float32(tot_a / (B * N) + tot_b / (B * M))


# revision 8
# speedup vs baseline: 1.3942x; 1.1783x over previous
"""Chamfer loss Trainium2 kernel.

Problem: B=8 batches of pred[4096,3] vs tgt[4096,3] point clouds.
chamfer = mean_n min_m ||p_n - t_m|| + mean_m min_n ||p_n - t_m||

Sharding: one batch element per NeuronCore (8 cores, SPMD).

Math: sqrt is monotonic, so mins are taken over *squared* distances and
only the final [4096] min-vectors get sqrt'd.  The squared distance
p2 + t2 - 2<p,t> is computed by a single K=5 augmented matmul:
  lhsT rows: [-2px, -2py, -2pz, 1, p2],  rhs rows: [tx, ty, tz, t2, 1]
so the PE writes sq[n,m] tiles straight into PSUM.  With K=5 the PE
array is addressed in 32-row strips (tile_position) so 4 matmuls run
concurrently.  DVE reduce_min drains PSUM; both orientations (row-min
and col-min) are separate matmul passes so each min is a free-axis
reduction.  Final sqrt+sum on device; the last [128,2]-per-core
reduction and the cross-core mean happen on host.
"""

import os
import numpy as np

B = 8
N = 4096  # pred points per batch
M = 4096  # tgt points per batch
D = 3
K = 5     # augmented contraction dim
P = 128   # partition block (rows per n-block)
F = 512   # matmul moving free dim (fp32 max / one PSUM bank)
NBLK = N // P   # 32
NCHK = M // F   # 8

_CACHE = {}


def _build_bass():
    import concourse.tile as tile
    from concourse import bacc, mybir

    f32 = mybir.dt.float32
    nc = bacc.Bacc(None, target_bir_lowering=False)

    f32r = mybir.dt.float32r
    wA = nc.dram_tensor("wA", [K, N], f32r, kind="ExternalInput")
    rA = nc.dram_tensor("rA", [K, M], f32r, kind="ExternalInput")
    wB = nc.dram_tensor("wB", [K, M], f32r, kind="ExternalInput")
    rB = nc.dram_tensor("rB", [K, N], f32r, kind="ExternalInput")
    out = nc.dram_tensor("out", [P, 2], f32, kind="ExternalOutput")

    f16 = mybir.dt.float16
    with tile.TileContext(nc) as tc:
        with (
            tc.tile_pool(name="inp", bufs=1) as inp_pool,
            tc.tile_pool(name="psum", bufs=2, space="PSUM") as psum_pool,
            tc.tile_pool(name="acc", bufs=1) as acc_pool,
            tc.tile_pool(name="cvt", bufs=3) as cvt_pool,
            tc.tile_pool(name="scr", bufs=2) as scr_pool,
        ):
            sums = acc_pool.tile([P, 2], f32, name="sums")
            for oi, (wd, rd) in enumerate([(wA, rA), (wB, rB)]):
                Wt = inp_pool.tile([P, N], f32r, name=f"Wt{oi}")
                Rt = inp_pool.tile([P, M], f32r, name=f"Rt{oi}")
                # replicate the K=5 augmented rows into the 4 PE row strips
                # (two DMA queues so the startup fill isn't serialized)
                for s in range(4):
                    nc.sync.dma_start(Wt[32 * s:32 * s + K, :], wd[:, :])
                    nc.gpsimd.dma_start(Rt[32 * s:32 * s + K, :], rd[:, :])

                # Per-block drain split (8 chunks of 512 per block):
                #  - DVE reduce_min's the first SD elems straight from PSUM
                #  - ACT (otherwise idle; it cannot min but CAN drain PSUM)
                #    converts the rest to fp16 in SBUF
                #  - DVE finishes the fp16 side with a TT-min tree at
                #    2 results/cycle, batched over G blocks per instruction
                #    to amortize per-op overhead
                SD = 2 * F            # direct-path elems per block (chunks 0-1)
                SC = 8 * F - SD       # 3072 fp16-path elems per block
                G = 4                 # blocks per batched tree
                rowdir = acc_pool.tile([P, NBLK], f32, name=f"rowdir_{oi}")
                rowm16 = acc_pool.tile([P, NBLK], f16, name=f"rowm16_{oi}")
                for g in range(NBLK // G):
                    cvt = cvt_pool.tile([P, G, SC], f16, tag="cvt")
                    for bi in range(G):
                        i = g * G + bi
                        # four 2-bank PSUM tiles per block: fine-grained
                        # release keeps the PE ahead of the drain engines
                        pss = []
                        for h in range(4):
                            ps = psum_pool.tile([P, 2 * F], f32, tag="ps",
                                                bufs=4)
                            for j in range(2):
                                c = h * 2 + j
                                s = c % 4
                                # float32r: fp32 bits via the replicated
                                # weight path — much faster than plain fp32
                                # when the moving free dim is >= 256
                                nc.tensor.matmul(
                                    ps[:, j * F:(j + 1) * F],
                                    Wt[32 * s:32 * s + K, i * P:(i + 1) * P],
                                    Rt[32 * s:32 * s + K, c * F:(c + 1) * F],
                                    start=True,
                                    stop=True,
                                    tile_position=(32 * s, 0),
                                )
                            pss.append(ps)
                        nc.vector.tensor_reduce(
                            rowdir[:, i:i + 1],
                            pss[0][:, :],
                            axis=mybir.AxisListType.X,
                            op=mybir.AluOpType.min,
                        )
                        for h in range(1, 4):
                            nc.scalar.copy(
                                cvt[:, bi, (h - 1) * 2 * F:h * 2 * F],
                                pss[h][:, :])
                    # batched fp16 min tree: SC -> SC/2 -> ... -> SC/16 -> 1
                    prev = cvt
                    w = SC
                    for lvl in range(4):
                        w //= 2
                        nxt = scr_pool.tile([P, G, w], f16, tag=f"l{lvl}",
                                            bufs=1 if lvl else 2)
                        nc.vector.tensor_tensor(
                            nxt[:, :, :], prev[:, :, 0:w], prev[:, :, w:2 * w],
                            op=mybir.AluOpType.min,
                        )
                        prev = nxt
                    nc.vector.tensor_reduce(
                        rowm16[:, g * G:(g + 1) * G], prev[:, :, :],
                        axis=mybir.AxisListType.X, op=mybir.AluOpType.min,
                    )
                # combine fp32/fp16 paths, clamp, sqrt, per-partition sum
                rowm16f = acc_pool.tile([P, NBLK], f32, name=f"rowm16f_{oi}")
                nc.vector.tensor_copy(rowm16f[:, :], rowm16[:, :])
                rowmin = acc_pool.tile([P, NBLK], f32, name=f"rowmin_{oi}")
                nc.vector.tensor_tensor(
                    rowmin[:, :], rowdir[:, :], rowm16f[:, :],
                    op=mybir.AluOpType.min,
                )
                nc.vector.tensor_scalar_max(rowmin[:, :], rowmin[:, :], 0.0)
                dist = acc_pool.tile([P, NBLK], f32, name=f"dist_{oi}")
                nc.scalar.sqrt(dist[:, :], rowmin[:, :])
                nc.vector.tensor_reduce(
                    sums[:, oi:oi + 1], dist[:, :],
                    axis=mybir.AxisListType.X, op=mybir.AluOpType.add,
                )
            nc.sync.dma_start(out[:, :], sums[:, :])

    nc.finalize()
    return nc


def _get_nc():
    if "nc" not in _CACHE:
        _CACHE["nc"] = _build_bass()
    return _CACHE["nc"]


def _augment(pts_w, pts_r):
    """Build (lhsT, rhs) aug matrices: sq = lhsT.T @ rhs."""
    ones_w = np.ones(pts_w.shape[0], np.float32)
    w2 = (pts_w * pts_w).sum(-1)
    r2 = (pts_r * pts_r).sum(-1)
    ones_r = np.ones(pts_r.shape[0], np.float32)
    lhsT = np.ascontiguousarray(
        np.stack([-2.0 * pts_w[:, 0], -2.0 * pts_w[:, 1], -2.0 * pts_w[:, 2],
                  ones_w, w2]).astype(np.float32))
    rhs = np.ascontiguousarray(
        np.stack([pts_r[:, 0], pts_r[:, 1], pts_r[:, 2], r2,
                  ones_r]).astype(np.float32))
    return lhsT, rhs


def _in_maps(predicted_points, target_points):
    maps = []
    for b in range(B):
        p = np.asarray(predicted_points[b], np.float32)
        t = np.asarray(target_points[b], np.float32)
        wA, rA = _augment(p, t)
        wB, rB = _augment(t, p)
        maps.append({"wA": wA, "rA": rA, "wB": wB, "rB": rB})
    return maps


def kernel(predicted_points, target_points):
    from concourse.bass_utils import run_bass_kernel_spmd

    nc = _get_nc()
    in_maps = _in_maps(predicted_points, target_points)
    trace = bool(int(os.environ.get("CHAMFER_TRACE", "0")))
    res = run_bass_kernel_spmd(
        nc, in_maps, core_ids=list(range(B)),
        trace=trace, trace_cores=[0] if trace else None,
    )
    _CACHE["last_result"] = res
    tot_a = 0.0
    tot_b = 0.0
    for b in range(B):
        o = res.results[b]["out"].astype(np.float64)
        tot_a += o[:, 0].sum()
        tot_b += o[:, 1].sum()
    return np.float32(tot_a / (B * N) + tot_b / (B * M))


# revision 24
# speedup vs baseline: 1.5341x; 1.1004x over previous
"""Chamfer loss Trainium2 kernel.

Problem: B=8 batches of pred[4096,3] vs tgt[4096,3] point clouds.
chamfer = mean_n min_m ||p_n - t_m|| + mean_m min_n ||p_n - t_m||

Sharding: one batch element per NeuronCore (8 cores, SPMD).

Math:
- sqrt is monotonic -> take mins over *squared* distances, sqrt only the
  final [4096] min-vectors.
- sq = p2 + t2 - 2<p,t> folded into ONE K=5 augmented matmul:
    lhsT rows: [-2px, -2py, -2pz, 1, p2], rhs rows: [tx, ty, tz, t2, 1]
  so the PE writes sq[n,m] tiles straight into PSUM (float32r = fast
  fp32 path).  K=5 < 32 so 4 matmuls run concurrently in separate
  32-row strips of the PE array (tile_position).
- Both orientations (row-min / col-min) are separate matmul passes,
  interleaved block-by-block so the pipeline never drains mid-kernel.
- PSUM can only be drained by DVE (0.96GHz) and ACT (1.2GHz) at ~1
  fp32/cycle/lane, so each block's 8 chunks split 50/50:
    * DVE reduce_min's chunks 0-3 exactly (two [128,1024] reduces).
    * ACT exp((q - sq)/T)-accumulates chunks 4-7 (ACT cannot min, but
      exp + row-sum IS a min: softmin).  The per-row shift q and
      temperature T = max(q,QFLOOR)/KAPPA come from the HOST (min over
      a 256-point subsample, O(N*256) prep) so ACT has no dependency
      on same-block DVE results - both engines free-run.
    * softmin bias ~ T*e^-gap/T is far below the float32r rounding
      noise; the QFLOOR floor stops noise-driven exponent overflow
      (rare overflow rows clamp to 0 harmlessly via the 1e19 sig cap).
- End-stage per orientation: min(r1, r2, q - T*ln(sum exp)) -> clamp
  -> sqrt -> per-partition sums; host does the final tiny reduction.
"""

import os
import numpy as np

B = 8
N = 4096  # pred points per batch
M = 4096  # tgt points per batch
D = 3
K = 5     # augmented contraction dim
P = 128   # partition block (rows per n-block)
F = 512   # matmul moving free dim (one PSUM bank of fp32)
NBLK = N // P   # 32
KAPPA = 80.0
QFLOOR = 0.02
NSAMP = 512     # host-side subsample size for the softmin shift

_CACHE = {}


def _build_bass():
    import concourse.tile as tile
    from concourse import bacc, mybir

    f32 = mybir.dt.float32
    f32r = mybir.dt.float32r
    bf16 = mybir.dt.bfloat16
    AX = mybir.AxisListType.X
    OP = mybir.AluOpType
    AF = mybir.ActivationFunctionType

    nc = bacc.Bacc(None, target_bir_lowering=False)

    wA = nc.dram_tensor("wA", [K, N], f32r, kind="ExternalInput")
    rA = nc.dram_tensor("rA", [K, M], f32r, kind="ExternalInput")
    wB = nc.dram_tensor("wB", [K, M], f32r, kind="ExternalInput")
    rB = nc.dram_tensor("rB", [K, N], f32r, kind="ExternalInput")
    # per-row softmin params from host, rows [scl, bias, T, q]
    pA = nc.dram_tensor("pA", [4, P, NBLK], f32, kind="ExternalInput")
    pB = nc.dram_tensor("pB", [4, P, NBLK], f32, kind="ExternalInput")
    out = nc.dram_tensor("out", [P, 2], f32, kind="ExternalOutput")

    with tile.TileContext(nc) as tc:
        with (
            tc.tile_pool(name="inp", bufs=1) as inp_pool,
            tc.tile_pool(name="psum", bufs=4, space="PSUM") as psum_pool,
            tc.tile_pool(name="acc", bufs=1) as acc_pool,
            tc.tile_pool(name="trash", bufs=3) as trash_pool,
        ):
            st = []
            # rowdir columns per block i: [r1A, r1B, r2A, r2B] at 4i
            rowdir = acc_pool.tile([P, 4 * NBLK], f32, name="rowdir")
            for oi, (wd, rd, pd) in enumerate(
                    [(wA, rA, pA), (wB, rB, pB)]):
                Wt = inp_pool.tile([P, N], f32r, name=f"Wt{oi}")
                Rt = inp_pool.tile([P, M], f32r, name=f"Rt{oi}")
                prm = inp_pool.tile([P, 4, NBLK], f32, name=f"prm{oi}")
                nc.sync.dma_start(prm[:, :, :], pd.rearrange("f p i -> p f i"))
                st.append(dict(
                    Wt=Wt, Rt=Rt, prm=prm,
                    esums=acc_pool.tile([P, NBLK, 2], f32,
                                        name=f"esums{oi}"),
                ))
            # input DMAs: orientation A first so compute starts early;
            # the K=5 augmented rows are replicated into all 4 PE strips
            for oi in range(2):
                for s in range(4):
                    nc.sync.dma_start(
                        st[oi]["Wt"][32 * s:32 * s + K, :],
                        (wA if oi == 0 else wB)[:, :])
                    nc.sync.dma_start(
                        st[oi]["Rt"][32 * s:32 * s + K, :],
                        (rA if oi == 0 else rB)[:, :])

            for i in range(NBLK):
                for oi in range(2):
                    S = st[oi]
                    Wt, Rt = S["Wt"], S["Rt"]
                    # 4 2-bank tiles/block: T0,T1 -> DVE exact reduce_min;
                    # T2,T3 -> ACT softmin (host-provided shift/temperature)
                    tiles = []
                    for h in range(4):
                        ps = psum_pool.tile([P, 2 * F], f32, tag="ps")
                        for j in range(2):
                            c = h * 2 + j
                            s = c % 4
                            nc.tensor.matmul(
                                ps[:, j * F:(j + 1) * F],
                                Wt[32 * s:32 * s + K, i * P:(i + 1) * P],
                                Rt[32 * s:32 * s + K, c * F:(c + 1) * F],
                                start=True,
                                stop=True,
                                tile_position=(32 * s, 0),
                            )
                        tiles.append(ps)
                    for h in range(2):
                        nc.vector.tensor_reduce(
                            rowdir[:, 4 * i + 2 * h + oi:
                                   4 * i + 2 * h + oi + 1],
                            tiles[h][:, :], axis=AX, op=OP.min)
                    for ei in range(2):
                        trash = trash_pool.tile([P, 2 * F], bf16, tag="tr")
                        nc.scalar.activation(
                            trash[:, :], tiles[2 + ei][:, :], AF.Exp,
                            bias=S["prm"][:, 1, i:i + 1],
                            scale=S["prm"][:, 0, i:i + 1],
                            accum_out=S["esums"][:, i, ei:ei + 1])

            # end-stage: softmin combine -> clamp -> sqrt -> row sums
            sums = acc_pool.tile([P, 2], f32, name="sums")
            for oi in range(2):
                S = st[oi]
                quads = rowdir.rearrange("p (i four) -> p i four", four=4)
                r1c = quads[:, :, oi]
                r2c = quads[:, :, 2 + oi]
                sig = acc_pool.tile([P, NBLK], f32, name=f"sig{oi}")
                nc.vector.tensor_reduce(sig[:, :], S["esums"][:, :, :],
                                        axis=AX, op=OP.add)
                # ACT Ln only accepts |x| <= 2^64: prescale by 2^-48 (the
                # +48*ln2 is folded back in below) and clamp into range
                nc.vector.tensor_scalar(sig[:, :], sig[:, :], 2.0 ** -64,
                                        1e-38, op0=OP.mult, op1=OP.max)
                nc.vector.tensor_scalar_min(sig[:, :], sig[:, :], 1e19)
                lns = acc_pool.tile([P, NBLK], f32, name=f"lns{oi}")
                nc.scalar.activation(lns[:, :], sig[:, :], AF.Ln)
                u = acc_pool.tile([P, NBLK], f32, name=f"u{oi}")
                nc.vector.scalar_tensor_tensor(
                    u[:, :], in0=lns[:, :], scalar=64.0 * float(np.log(2.0)),
                    in1=S["prm"][:, 2, :], op0=OP.add, op1=OP.mult)
                sm = acc_pool.tile([P, NBLK], f32, name=f"sm{oi}")
                nc.vector.tensor_tensor(sm[:, :], S["prm"][:, 3, :], u[:, :],
                                        op=OP.subtract)
                nc.vector.tensor_tensor(sm[:, :], sm[:, :], r1c, op=OP.min)
                nc.vector.tensor_tensor(sm[:, :], sm[:, :], r2c, op=OP.min)
                nc.vector.tensor_scalar_max(sm[:, :], sm[:, :], 0.0)
                dist = acc_pool.tile([P, NBLK], f32, name=f"dist{oi}")
                nc.scalar.sqrt(dist[:, :], sm[:, :])
                nc.vector.tensor_reduce(sums[:, oi:oi + 1], dist[:, :],
                                        axis=AX, op=OP.add)
            nc.sync.dma_start(out[:, :], sums[:, :])

    nc.finalize()
    return nc


def _get_nc():
    if "nc" not in _CACHE:
        _CACHE["nc"] = _build_bass()
    return _CACHE["nc"]


def _augment(pts_w, pts_r):
    """Build (lhsT, rhs) aug matrices: sq = lhsT.T @ rhs."""
    ones_w = np.ones(pts_w.shape[0], np.float32)
    w2 = (pts_w * pts_w).sum(-1)
    r2 = (pts_r * pts_r).sum(-1)
    ones_r = np.ones(pts_r.shape[0], np.float32)
    lhsT = np.ascontiguousarray(
        np.stack([-2.0 * pts_w[:, 0], -2.0 * pts_w[:, 1], -2.0 * pts_w[:, 2],
                  ones_w, w2]).astype(np.float32))
    rhs = np.ascontiguousarray(
        np.stack([pts_r[:, 0], pts_r[:, 1], pts_r[:, 2], r2,
                  ones_r]).astype(np.float32))
    return lhsT, rhs


def _shift_params(pts_w, pts_r):
    """Host-side softmin shift: q[n] = min over a subsample of targets."""
    step = max(1, pts_r.shape[0] // NSAMP)
    sub = pts_r[::step]
    d = ((pts_w[:, None, :] - sub[None, :, :]) ** 2).sum(-1)
    q = d.min(1).astype(np.float32)                      # [n], >= true min
    mx = np.maximum(q, np.float32(QFLOOR))
    T = mx / np.float32(KAPPA)
    scl = (-np.float32(KAPPA) / mx).astype(np.float32)
    bias = (-scl * q).astype(np.float32)
    arr = np.stack([scl, bias, T, q])                    # [4, n]
    return np.ascontiguousarray(
        arr.reshape(4, NBLK, P).transpose(0, 2, 1))      # [4, P, NBLK]


def _in_maps(predicted_points, target_points):
    maps = []
    for b in range(B):
        p = np.asarray(predicted_points[b], np.float32)
        t = np.asarray(target_points[b], np.float32)
        wA, rA = _augment(p, t)
        wB, rB = _augment(t, p)
        maps.append({"wA": wA, "rA": rA, "wB": wB, "rB": rB,
                     "pA": _shift_params(p, t), "pB": _shift_params(t, p)})
    return maps


def kernel(predicted_points, target_points):
    from concourse.bass_utils import run_bass_kernel_spmd

    nc = _get_nc()
    in_maps = _in_maps(predicted_points, target_points)
    trace = bool(int(os.environ.get("CHAMFER_TRACE", "0")))
    res = run_bass_kernel_spmd(
        nc, in_maps, core_ids=list(range(B)),
        trace=trace, trace_cores=[0] if trace else None,
    )
    _CACHE["last_result"] = res
    tot_a = 0.0
    tot_b = 0.0
    for b in range(B):
        o = res.results[b]["out"].astype(np.float64)
        tot_a += o[:, 0].sum()
        tot_b += o[:, 1].sum()
    return np.float32(tot_a / (B * N) + tot_b / (B * M))
